# revision 35
# baseline (speedup 1.0000x reference)
"""CIN (xDeepFM CompressedInteractionNetwork) forward on 8 TRN2 NeuronCores.

Strategy (pure data parallelism, hardcoded from the problem spec):
  - batch 4096 -> 512 per core; 64 tiles of 8 batch elements; matmul free
    dim = 512 columns = (8 batch x 64 embed).
  - layer l: out[o, col] = relu( sum_c W[o,c] * z[c, col] + b[o] ) where
    z[f*Hin+j, col] = x0[f, col] * h[j, col].  z is materialized as bf16
    tensor_tensor multiplies (x0 rows partition-broadcast via DMA, h
    broadcast along a stride-0 free dim).
  - matmuls in bf16 (full PE rate), fp32 PSUM; ScalarE applies bias+relu
    out of PSUM; DVE pools (sum over embed); final FC on host.

Perf structure:
  - software-pipelined PE stream: per iteration i the tensor engine runs
    L0(i), L2(i-1), L1(i); each layer's h-producing chain (m=1) first.
    All PE dependencies are then ~a full layer old -> no 8us z-build
    stalls (the v1 baseline lost 1.0ms/core to 128 such gaps and sat at
    the 1.2GHz p-state; a dense stream sustains 2.4GHz).
  - L0 exploits z0 = x (x) x symmetry: the 1024-channel contraction
    folds to 528 unordered pairs (symmetrized W0), padded to 640=5x128
    -> 10 instead of 16 matmuls per tile. Pair operands are host-
    gathered arrays, plain per-partition-contiguous DMA loads.
  - z0-build on the Pool engine (gpsimd), z1/z2 builds + pooling reduces
    on DVE (r tiles bf16 for 2x DVE reduce rate).
  - host pre-lays x out per-tile ([tile, f, b, e]) so every partition-
    broadcast DMA reads a contiguous 16KB block -> 16KB descriptors
    instead of 1KB (v1: 16 DMA engines x 1.65ms busy; now ~0.6ms).
  - warmup: tile-1 L0 primed into iteration 0, startup DMAs ordered by
    first-use time; first matmul issues at ~10us.
"""

import sys

sys.path.insert(0, "/opt/trn_rl_repo")

import numpy as np
import ml_dtypes
from contextlib import ExitStack

N_CORES = 8
B = 4096
F = 32
E = 64
BC = B // N_CORES  # 512 batch elements per core
NB = 8             # batch elements per tile
COLS = NB * E      # 512 matmul columns per tile
NT = BC // NB      # 64 tiles per core
O = 256            # conv out channels per layer

_CACHE = {}


def _build(n_tiles=NT):
    import concourse.bass as bass  # noqa: F401
    import concourse.mybir as mybir
    import concourse.tile as tile
    from concourse import bacc

    dt = mybir.dt
    AF = mybir.ActivationFunctionType
    ALU = mybir.AluOpType
    AX = mybir.AxisListType

    nc = bacc.Bacc("TRN2", target_bir_lowering=False, debug=False,
                   num_devices=N_CORES)

    # x pre-laid out per tile: xtile[t, f, (b e)] so broadcast DMAs read
    # contiguous blocks. L0 uses the z0 symmetry (x (x) x): 1024 channels
    # collapse to 528 unordered pairs, padded to 640 = 5x128; xpa/xpb are
    # host-gathered pair operands in [t, p, (g c)] per-partition layout.
    xtile = nc.declare_dram_parameter("xtile", [n_tiles, F, COLS],
                                      dt.bfloat16, isOutput=False)
    xpa = nc.declare_dram_parameter("xpa", [n_tiles, 128, 5 * COLS],
                                    dt.bfloat16, isOutput=False)
    xpb = nc.declare_dram_parameter("xpb", [n_tiles, 128, 5 * COLS],
                                    dt.bfloat16, isOutput=False)
    # weights host-prepped to [p, g, o] (per-partition contiguous blocks);
    # w0t is the symmetrized+padded L0 weight (640 channels).
    w0t = nc.declare_dram_parameter("w0t", [128, 5 * O], dt.bfloat16, isOutput=False)
    w1t = nc.declare_dram_parameter("w1t", [128, 32 * O], dt.bfloat16, isOutput=False)
    # L2 runs fp8e4 DoubleRow (2 contraction groups per matmul, 0.5 cyc/row)
    w2t = nc.declare_dram_parameter("w2t", [128, 32 * O], dt.float8e4, isOutput=False)
    b0 = nc.declare_dram_parameter("b0", [O], dt.float32, isOutput=False)
    b1 = nc.declare_dram_parameter("b1", [O], dt.float32, isOutput=False)
    b2 = nc.declare_dram_parameter("b2", [O], dt.float32, isOutput=False)
    pout = nc.declare_dram_parameter("pout", [4, 128, n_tiles * NB],
                                     dt.float32, isOutput=True)

    with ExitStack() as ctx:
        tc = ctx.enter_context(tile.TileContext(nc))
        const = ctx.enter_context(tc.tile_pool(name="const", bufs=1))

        # ---- persistent weights / biases (DMAs interleaved below so tile-0
        #      prefetch isn't stuck behind 4.5MB of weights) ----
        lw0 = const.tile([128, 5, O], dt.bfloat16)       # w chunk [c=128g+p]
        # lw1/lw2 split by out-half m so each chain's weights can be DMA'd
        # separately in first-use order: [p, m, g, o']
        lw1 = const.tile([128, 2, 32, 128], dt.bfloat16)
        lw2 = const.tile([128, 2, 32, 128], dt.float8e4)
        bias0 = const.tile([128, 2], dt.float32)
        bias1 = const.tile([128, 2], dt.float32)
        bias2 = const.tile([128, 2], dt.float32)

        # pooled accumulators [o_chunk 128, batch 512]
        P0 = const.tile([128, n_tiles * NB], dt.float32)
        P1 = const.tile([128, n_tiles * NB], dt.float32)
        P2a = const.tile([128, n_tiles * NB], dt.float32)
        P2b = const.tile([128, n_tiles * NB], dt.float32)

        # ---- rotating pools ----
        xr_pool = ctx.enter_context(tc.tile_pool(name="xr", bufs=3))
        za_pool = ctx.enter_context(tc.tile_pool(name="za", bufs=2))
        zb_pool = ctx.enter_context(tc.tile_pool(name="zb", bufs=2))
        z0_pool = ctx.enter_context(tc.tile_pool(name="z0", bufs=2))
        z1_pool = ctx.enter_context(tc.tile_pool(name="z1", bufs=2))
        z2_pool = ctx.enter_context(tc.tile_pool(name="z2", bufs=2))
        h_pool = ctx.enter_context(tc.tile_pool(name="h", bufs=3))
        r_pool = ctx.enter_context(tc.tile_pool(name="r", bufs=4))
        psum_pool = ctx.enter_context(tc.tile_pool(name="ps", bufs=6, space="PSUM"))

        # per-tile state carried across pipeline stages
        xrh = [None] * n_tiles   # [2 x tile [128,16,COLS]]
        z0t = [None] * n_tiles
        z2t = [None] * n_tiles
        za_t = [None] * n_tiles
        zb_t = [None] * n_tiles

        def emit_dma_pair(t):
            """Prefetch tile t's L0 pair operands (gate z0(t) -> L0(t))."""
            za = za_pool.tile([128, 5, COLS], dt.bfloat16)
            nc.sync.dma_start(za[:].rearrange("p g c -> p (g c)"), xpa.ap()[t])
            za_t[t] = za
            zb = zb_pool.tile([128, 5, COLS], dt.bfloat16)
            nc.sync.dma_start(zb[:].rearrange("p g c -> p (g c)"), xpb.ap()[t])
            zb_t[t] = zb

        def emit_dma_xr(t):
            """Prefetch tile t's replicated x rows (feed z1/z2 builds)."""
            halves = []
            for half in range(2):
                xr_t = xr_pool.tile([128, 16, COLS], dt.bfloat16,
                                    name=f"xr{half}", tag="xr")
                src = xtile.ap()[t, half * 16:(half + 1) * 16, :] \
                    .unsqueeze(0).broadcast_to([128, 16, COLS])
                nc.sync.dma_start(xr_t[:], src)
                halves.append(xr_t)
            xrh[t] = halves

        def emit_dma(t):
            emit_dma_pair(t)
            emit_dma_xr(t)

        def emit_z0(t, eng=None):
            """z0(t) on the Pool engine (keeps DVE free for z1/z2)."""
            z0 = z0_pool.tile([128, 5, COLS], dt.bfloat16)
            (eng or nc.gpsimd).tensor_tensor(
                z0[:], za_t[t][:], zb_t[t][:], ALU.mult)
            z0t[t] = z0

        def reduce_into(P, t, r_t):
            nc.vector.tensor_reduce(
                P[:, t * NB:(t + 1) * NB],
                r_t[:].rearrange("p (b e) -> p b e", e=E), AX.X, ALU.add)

        # ---- preamble: startup DMAs ordered by first-use time ----
        nc.sync.dma_start(lw0[:], w0t.ap().rearrange("p (g o) -> p g o", o=O))
        nc.sync.dma_start(bias0[:], b0.ap().rearrange("(m p) -> p m", p=128))
        emit_dma_pair(0)
        emit_z0(0, eng=nc.vector)   # DVE: fast; gates very first matmul
        if n_tiles > 1:
            emit_dma_pair(1)
            emit_z0(1, eng=nc.vector)  # gates the iter-0 primer L0(1)
        emit_dma_xr(0)
        w1v = w1t.ap().rearrange("p (m x) -> p m x", m=2)
        nc.sync.dma_start(lw1[:, 1].rearrange("p g o -> p (g o)"), w1v[:, 1])
        nc.sync.dma_start(bias1[:], b1.ap().rearrange("(m p) -> p m", p=128))
        nc.sync.dma_start(lw1[:, 0].rearrange("p g o -> p (g o)"), w1v[:, 0])
        if n_tiles > 1:
            emit_dma_xr(1)
        w2v = w2t.ap().rearrange("p (m x) -> p m x", m=2)
        nc.sync.dma_start(lw2[:, 0].rearrange("p g o -> p (g o)"), w2v[:, 0])
        nc.sync.dma_start(bias2[:], b2.ap().rearrange("(m p) -> p m", p=128))
        nc.sync.dma_start(lw2[:, 1].rearrange("p g o -> p (g o)"), w2v[:, 1])

        ps0_of = {}

        def emit_L0(t):
            """PE chains of L0(t); m=1 (h-half) first."""
            ps0 = {m: psum_pool.tile([128, COLS], dt.float32,
                                     name=f"ps0{m}", tag="ps")
                   for m in (1, 0)}
            for m in (1, 0):
                for g in range(5):
                    nc.tensor.matmul(
                        ps0[m][:], lw0[:, g, m * 128:(m + 1) * 128],
                        z0t[t][:, g, :], start=(g == 0), stop=(g == 4))
            ps0_of[t] = ps0

        for i in range(n_tiles + 1):
            if i + 1 < n_tiles and za_t[i + 1] is None:
                emit_dma(i + 1)

            if i < n_tiles:
                if i not in ps0_of:
                    emit_L0(i)
                ps0 = ps0_of.pop(i)
                h1 = h_pool.tile([128, COLS], dt.bfloat16, name="h1", tag="h")
                nc.scalar.activation(h1[:], ps0[1][:], AF.Relu, bias=bias0[:, 1:2])
                r0 = r_pool.tile([128, COLS], dt.bfloat16, name="r0", tag="r")
                nc.scalar.activation(r0[:], ps0[0][:], AF.Relu, bias=bias0[:, 0:1])

                # -- DVE: z1(i) halves --
                z1h = []
                for half in range(2):
                    z_t = z1_pool.tile([128, 16, COLS], dt.bfloat16,
                                       name=f"z1{half}", tag="z1")
                    nc.vector.tensor_tensor(
                        z_t[:], xrh[i][half][:],
                        h1[:].unsqueeze(1).broadcast_to([128, 16, COLS]),
                        ALU.mult)
                    z1h.append(z_t)

                if i == 0 and n_tiles > 1:
                    # primer: fill the pipeline-warmup PE gap (L1(0) waits
                    # on z1(0)) with tile 1's L0 chains (z0(1): preamble).
                    emit_L0(1)

            if i >= 1:
                # -- PE: L2(i-1) --
                c = i - 1
                ps2 = {m: psum_pool.tile([128, COLS], dt.float32,
                                         name=f"ps2{m}", tag="ps")
                       for m in (0, 1)}
                for m in (0, 1):
                    for half in range(2):
                        for g2 in range(8):
                            nc.tensor.matmul(
                                ps2[m][:],
                                lw2[:, m, half * 16 + 2 * g2:half * 16 + 2 * g2 + 2, :],
                                z2t[c][half][:, 2 * g2:2 * g2 + 2, :],
                                start=(half == 0 and g2 == 0),
                                stop=(half == 1 and g2 == 7),
                                perf_mode=mybir.MatmulPerfMode.DoubleRow)
                r2a = r_pool.tile([128, COLS], dt.bfloat16, name="r2a", tag="r")
                nc.scalar.activation(r2a[:], ps2[0][:], AF.Relu, bias=bias2[:, 0:1])
                r2b = r_pool.tile([128, COLS], dt.bfloat16, name="r2b", tag="r")
                nc.scalar.activation(r2b[:], ps2[1][:], AF.Relu, bias=bias2[:, 1:2])

            # -- Pool: z0(i+1) (deps: tile i+1 DMAs only) --
            if i + 1 < n_tiles and z0t[i + 1] is None:
                emit_z0(i + 1)

            if i < n_tiles:
                # -- PE: L1(i); m=1 (h-half) first --
                ps1 = {m: psum_pool.tile([128, COLS], dt.float32,
                                         name=f"ps1{m}", tag="ps")
                       for m in (1, 0)}
                for m in (1, 0):
                    for half in range(2):
                        for g in range(16):
                            nc.tensor.matmul(
                                ps1[m][:],
                                lw1[:, m, half * 16 + g, :],
                                z1h[half][:, g, :],
                                start=(half == 0 and g == 0),
                                stop=(half == 1 and g == 15))
                h2 = h_pool.tile([128, COLS], dt.bfloat16, name="h2", tag="h")
                nc.scalar.activation(h2[:], ps1[1][:], AF.Relu, bias=bias1[:, 1:2])
                r1 = r_pool.tile([128, COLS], dt.bfloat16, name="r1", tag="r")
                nc.scalar.activation(r1[:], ps1[0][:], AF.Relu, bias=bias1[:, 0:1])

                # -- DVE: z2(i) halves (fp8 for the DoubleRow L2 matmuls) --
                z2h = []
                for half in range(2):
                    z_t = z2_pool.tile([128, 16, COLS], dt.float8e4,
                                       name=f"z2{half}", tag="z2")
                    nc.vector.tensor_tensor(
                        z_t[:], xrh[i][half][:],
                        h2[:].unsqueeze(1).broadcast_to([128, 16, COLS]),
                        ALU.mult)
                    z2h.append(z_t)
                z2t[i] = z2h

                # -- DVE: pooling reduces --
                reduce_into(P0, i, r0)
                if i >= 1:
                    reduce_into(P2a, i - 1, r2a)
                    reduce_into(P2b, i - 1, r2b)
                reduce_into(P1, i, r1)
                if i == n_tiles - 1:
                    # P0/P1 are complete now; overlap their write-out with
                    # the final L2 tile
                    nc.sync.dma_start(pout.ap()[0], P0[:])
                    nc.sync.dma_start(pout.ap()[1], P1[:])
            else:
                reduce_into(P2a, i - 1, r2a)
                reduce_into(P2b, i - 1, r2b)

        # ---- ship remaining accumulators; tiny FC happens on host ----
        nc.sync.dma_start(pout.ap()[2], P2a[:])
        nc.sync.dma_start(pout.ap()[3], P2b[:])

    nc.compile()
    return nc


def _pair_indices():
    """Unordered-pair enumeration for the symmetric L0 contraction:
    32 diagonal pairs first, then the 496 f1<f2 pairs (total 528)."""
    ia = [f for f in range(F)]
    ib = [f for f in range(F)]
    for f1 in range(F):
        for f2 in range(f1 + 1, F):
            ia.append(f1)
            ib.append(f2)
    return np.asarray(ia, np.int64), np.asarray(ib, np.int64)


def _prep_inputs(x, w0, b0, w1, b1, w2, b2, fc_w, fc_b):
    bf16 = ml_dtypes.bfloat16
    xb = np.asarray(x, dtype=np.float32).astype(bf16)

    def wprep(w, G):
        # w [O, c] with c = 128*g + p  ->  [p, (g, o)] contiguous per p
        wt = np.asarray(w, np.float32).T.reshape(G, 128, O)  # [g, p, o]
        return np.ascontiguousarray(
            wt.transpose(1, 0, 2).reshape(128, G * O)).astype(bf16)

    def wprep_m(w, G, dtype=None):
        # -> [p, (m, g, o')]: out-half-major so each m-chain DMAs separately
        wt = np.asarray(w, np.float32).T.reshape(G, 128, 2, 128)  # [g,p,m,o']
        return np.ascontiguousarray(
            wt.transpose(1, 2, 0, 3).reshape(128, G * O)).astype(dtype or bf16)

    # L0 symmetrization: channel c = 32*f1 + f2, z0[c] = x[f1]*x[f2] is
    # symmetric -> fold to 528 unordered pairs (pad to 640 = 5*128).
    ia, ib = _pair_indices()
    w0f = np.asarray(w0, np.float32).reshape(O, F, F)
    w0s = np.zeros((O, 640), np.float32)
    npair = len(ia)  # 528
    w0s[:, :npair] = w0f[:, ia, ib]
    off = ia != ib
    w0s[:, :npair][:, off] += w0f[:, ib[off], ia[off]]
    w0t = wprep(w0s, 5)
    w1t = wprep_m(w1, 32)
    w2t = wprep_m(w2, 32, dtype=ml_dtypes.float8_e4m3)
    common = {
        "w0t": w0t, "w1t": w1t, "w2t": w2t,
        "b0": np.ascontiguousarray(np.asarray(b0, np.float32)),
        "b1": np.ascontiguousarray(np.asarray(b1, np.float32)),
        "b2": np.ascontiguousarray(np.asarray(b2, np.float32)),
    }
    # pad pair index lists to 640 (padded channels have zero weight)
    iap = np.zeros(640, np.int64); iap[:len(ia)] = ia
    ibp = np.zeros(640, np.int64); ibp[:len(ib)] = ib
    in_maps = []
    for c in range(N_CORES):
        m = dict(common)
        xc = xb[c * BC:(c + 1) * BC]                     # [BC, F, E]
        # xtile[t, f, (b e)] = x[8t+b, f, e]
        xt = np.ascontiguousarray(
            xc.reshape(NT, NB, F, E).transpose(0, 2, 1, 3).reshape(NT, F, COLS))
        m["xtile"] = xt
        # xpa/xpb[t, p, (g c)] = xtile[t, idx[128g+p], c]  (pair operands)
        for nm, idx in (("xpa", iap), ("xpb", ibp)):
            g = xt[:, idx, :]                            # [NT, 640, COLS]
            m[nm] = np.ascontiguousarray(
                g.reshape(NT, 5, 128, COLS).transpose(0, 2, 1, 3)
                 .reshape(NT, 128, 5 * COLS))
        in_maps.append(m)
    return in_maps


def kernel(x, w0, b0, w1, b1, w2, b2, fc_w, fc_b, **kw):
    from concourse.bass_utils import run_bass_kernel_spmd

    if "nc" not in _CACHE:
        _CACHE["nc"] = _build()
    nc = _CACHE["nc"]
    in_maps = _prep_inputs(x, w0, b0, w1, b1, w2, b2, fc_w, fc_b)
    res = run_bass_kernel_spmd(nc, in_maps, list(range(N_CORES)))
    fcw = np.asarray(fc_w, np.float32).reshape(4, 128)
    ys = []
    for c in range(N_CORES):
        p = res.results[c]["pout"]  # [4, 128, BC]
        ys.append(np.einsum('cp,cpb->b', fcw, p.astype(np.float32)))
    out = np.concatenate(ys).reshape(B, 1).astype(np.float32)
    out = out + np.asarray(fc_b, np.float32).reshape(1, 1)
    return out


# revision 36
# speedup vs baseline: 1.2304x; 1.2304x over previous
"""CIN (xDeepFM CompressedInteractionNetwork) forward on 8 TRN2 NeuronCores.

Strategy (pure data parallelism, hardcoded from the problem spec):
  - batch 4096 -> 512 per core; 64 tiles of 8 batch elements; matmul free
    dim = 512 columns = (8 batch x 64 embed).
  - layer l: out[o, col] = relu( sum_c W[o,c] * z[c, col] + b[o] ) where
    z[f*Hin+j, col] = x0[f, col] * h[j, col].  z is materialized as bf16
    tensor_tensor multiplies (x0 rows partition-broadcast via DMA, h
    broadcast along a stride-0 free dim).
  - matmuls in bf16 (full PE rate), fp32 PSUM; ScalarE applies bias+relu
    out of PSUM; DVE pools (sum over embed); final FC on host.

Perf structure:
  - software-pipelined PE stream: per iteration i the tensor engine runs
    L0(i), L2(i-1), L1(i); each layer's h-producing chain (m=1) first.
    All PE dependencies are then ~a full layer old -> no 8us z-build
    stalls (the v1 baseline lost 1.0ms/core to 128 such gaps and sat at
    the 1.2GHz p-state; a dense stream sustains 2.4GHz).
  - L0 exploits z0 = x (x) x symmetry: the 1024-channel contraction
    folds to 528 unordered pairs (symmetrized W0), padded to 640=5x128
    -> 10 instead of 16 matmuls per tile. Pair operands are host-
    gathered arrays, plain per-partition-contiguous DMA loads.
  - z0-build on the Pool engine (gpsimd), z1/z2 builds + pooling reduces
    on DVE (r tiles bf16 for 2x DVE reduce rate).
  - host pre-lays x out per-tile ([tile, f, b, e]) so every partition-
    broadcast DMA reads a contiguous 16KB block -> 16KB descriptors
    instead of 1KB (v1: 16 DMA engines x 1.65ms busy; now ~0.6ms).
  - warmup: tile-1 L0 primed into iteration 0, startup DMAs ordered by
    first-use time; first matmul issues at ~10us.
"""

import sys

sys.path.insert(0, "/opt/trn_rl_repo")

import numpy as np
import ml_dtypes
from contextlib import ExitStack

N_CORES = 8
B = 4096
F = 32
E = 64
BC = B // N_CORES  # 512 batch elements per core
NB = 8             # batch elements per tile
COLS = NB * E      # 512 matmul columns per tile
NT = BC // NB      # 64 tiles per core
O = 256            # conv out channels per layer

_CACHE = {}


def _build(n_tiles=NT):
    import concourse.bass as bass  # noqa: F401
    import concourse.mybir as mybir
    import concourse.tile as tile
    from concourse import bacc

    dt = mybir.dt
    AF = mybir.ActivationFunctionType
    ALU = mybir.AluOpType
    AX = mybir.AxisListType

    nc = bacc.Bacc("TRN2", target_bir_lowering=False, debug=False,
                   num_devices=N_CORES)

    # x pre-laid out per tile: xtile[t, f, (b e)] so broadcast DMAs read
    # contiguous blocks. L0 uses the z0 symmetry (x (x) x): 1024 channels
    # collapse to 528 unordered pairs, padded to 640 = 5x128; xpa/xpb are
    # host-gathered pair operands in [t, p, (g c)] per-partition layout.
    xtile = nc.declare_dram_parameter("xtile", [n_tiles, F, COLS],
                                      dt.bfloat16, isOutput=False)
    xpa = nc.declare_dram_parameter("xpa", [n_tiles, 128, 5 * COLS],
                                    dt.bfloat16, isOutput=False)
    xpb = nc.declare_dram_parameter("xpb", [n_tiles, 128, 5 * COLS],
                                    dt.bfloat16, isOutput=False)
    # weights host-prepped to [p, g, o] (per-partition contiguous blocks);
    # w0t is the symmetrized+padded L0 weight (640 channels).
    w0t = nc.declare_dram_parameter("w0t", [128, 5 * O], dt.bfloat16, isOutput=False)
    w1t = nc.declare_dram_parameter("w1t", [128, 32 * O], dt.bfloat16, isOutput=False)
    w2t = nc.declare_dram_parameter("w2t", [128, 32 * O], dt.bfloat16, isOutput=False)
    b0 = nc.declare_dram_parameter("b0", [O], dt.float32, isOutput=False)
    b1 = nc.declare_dram_parameter("b1", [O], dt.float32, isOutput=False)
    b2 = nc.declare_dram_parameter("b2", [O], dt.float32, isOutput=False)
    pout = nc.declare_dram_parameter("pout", [4, 128, n_tiles * NB],
                                     dt.float32, isOutput=True)

    with ExitStack() as ctx:
        tc = ctx.enter_context(tile.TileContext(nc))
        const = ctx.enter_context(tc.tile_pool(name="const", bufs=1))

        # ---- persistent weights / biases (DMAs interleaved below so tile-0
        #      prefetch isn't stuck behind 4.5MB of weights) ----
        lw0 = const.tile([128, 5, O], dt.bfloat16)       # w chunk [c=128g+p]
        # lw1/lw2 split by out-half m so each chain's weights can be DMA'd
        # separately in first-use order: [p, m, g, o']
        lw1 = const.tile([128, 2, 32, 128], dt.bfloat16)
        lw2 = const.tile([128, 2, 32, 128], dt.bfloat16)
        bias0 = const.tile([128, 2], dt.float32)
        bias1 = const.tile([128, 2], dt.float32)
        bias2 = const.tile([128, 2], dt.float32)

        # pooled accumulators [o_chunk 128, batch 512]
        P0 = const.tile([128, n_tiles * NB], dt.float32)
        P1 = const.tile([128, n_tiles * NB], dt.float32)
        P2a = const.tile([128, n_tiles * NB], dt.float32)
        P2b = const.tile([128, n_tiles * NB], dt.float32)

        # ---- rotating pools ----
        xr_pool = ctx.enter_context(tc.tile_pool(name="xr", bufs=3))
        za_pool = ctx.enter_context(tc.tile_pool(name="za", bufs=2))
        zb_pool = ctx.enter_context(tc.tile_pool(name="zb", bufs=2))
        z0_pool = ctx.enter_context(tc.tile_pool(name="z0", bufs=2))
        z1_pool = ctx.enter_context(tc.tile_pool(name="z1", bufs=2))
        z2_pool = ctx.enter_context(tc.tile_pool(name="z2", bufs=2))
        h_pool = ctx.enter_context(tc.tile_pool(name="h", bufs=3))
        r_pool = ctx.enter_context(tc.tile_pool(name="r", bufs=4))
        psum_pool = ctx.enter_context(tc.tile_pool(name="ps", bufs=6, space="PSUM"))

        # per-tile state carried across pipeline stages
        xrh = [None] * n_tiles   # [2 x tile [128,16,COLS]]
        z0t = [None] * n_tiles
        z2t = [None] * n_tiles
        za_t = [None] * n_tiles
        zb_t = [None] * n_tiles

        def emit_dma_pair(t):
            """Prefetch tile t's L0 pair operands (gate z0(t) -> L0(t))."""
            za = za_pool.tile([128, 5, COLS], dt.bfloat16)
            nc.sync.dma_start(za[:].rearrange("p g c -> p (g c)"), xpa.ap()[t])
            za_t[t] = za
            zb = zb_pool.tile([128, 5, COLS], dt.bfloat16)
            nc.sync.dma_start(zb[:].rearrange("p g c -> p (g c)"), xpb.ap()[t])
            zb_t[t] = zb

        def emit_dma_xr(t):
            """Prefetch tile t's replicated x rows (feed z1/z2 builds)."""
            halves = []
            for half in range(2):
                xr_t = xr_pool.tile([128, 16, COLS], dt.bfloat16,
                                    name=f"xr{half}", tag="xr")
                src = xtile.ap()[t, half * 16:(half + 1) * 16, :] \
                    .unsqueeze(0).broadcast_to([128, 16, COLS])
                nc.sync.dma_start(xr_t[:], src)
                halves.append(xr_t)
            xrh[t] = halves

        def emit_dma(t):
            emit_dma_pair(t)
            emit_dma_xr(t)

        def emit_z0(t, eng=None):
            """z0(t) on the Pool engine (keeps DVE free for z1/z2)."""
            z0 = z0_pool.tile([128, 5, COLS], dt.bfloat16)
            (eng or nc.gpsimd).tensor_tensor(
                z0[:], za_t[t][:], zb_t[t][:], ALU.mult)
            z0t[t] = z0

        def reduce_into(P, t, r_t):
            nc.vector.tensor_reduce(
                P[:, t * NB:(t + 1) * NB],
                r_t[:].rearrange("p (b e) -> p b e", e=E), AX.X, ALU.add)

        # ---- preamble: startup DMAs ordered by first-use time ----
        nc.sync.dma_start(lw0[:], w0t.ap().rearrange("p (g o) -> p g o", o=O))
        nc.sync.dma_start(bias0[:], b0.ap().rearrange("(m p) -> p m", p=128))
        emit_dma_pair(0)
        emit_z0(0, eng=nc.vector)   # DVE: fast; gates very first matmul
        if n_tiles > 1:
            emit_dma_pair(1)
            emit_z0(1, eng=nc.vector)  # gates the iter-0 primer L0(1)
        emit_dma_xr(0)
        w1v = w1t.ap().rearrange("p (m x) -> p m x", m=2)
        nc.sync.dma_start(lw1[:, 1].rearrange("p g o -> p (g o)"), w1v[:, 1])
        nc.sync.dma_start(bias1[:], b1.ap().rearrange("(m p) -> p m", p=128))
        nc.sync.dma_start(lw1[:, 0].rearrange("p g o -> p (g o)"), w1v[:, 0])
        if n_tiles > 1:
            emit_dma_xr(1)
        w2v = w2t.ap().rearrange("p (m x) -> p m x", m=2)
        nc.sync.dma_start(lw2[:, 0].rearrange("p g o -> p (g o)"), w2v[:, 0])
        nc.sync.dma_start(bias2[:], b2.ap().rearrange("(m p) -> p m", p=128))
        nc.sync.dma_start(lw2[:, 1].rearrange("p g o -> p (g o)"), w2v[:, 1])

        ps0_of = {}

        def emit_L0(t):
            """PE chains of L0(t); m=1 (h-half) first."""
            ps0 = {m: psum_pool.tile([128, COLS], dt.float32,
                                     name=f"ps0{m}", tag="ps")
                   for m in (1, 0)}
            for m in (1, 0):
                for g in range(5):
                    nc.tensor.matmul(
                        ps0[m][:], lw0[:, g, m * 128:(m + 1) * 128],
                        z0t[t][:, g, :], start=(g == 0), stop=(g == 4))
            ps0_of[t] = ps0

        for i in range(n_tiles + 1):
            if i + 1 < n_tiles and za_t[i + 1] is None:
                emit_dma(i + 1)

            if i < n_tiles:
                if i not in ps0_of:
                    emit_L0(i)
                ps0 = ps0_of.pop(i)
                h1 = h_pool.tile([128, COLS], dt.bfloat16, name="h1", tag="h")
                nc.scalar.activation(h1[:], ps0[1][:], AF.Relu, bias=bias0[:, 1:2])
                r0 = r_pool.tile([128, COLS], dt.bfloat16, name="r0", tag="r")
                nc.scalar.activation(r0[:], ps0[0][:], AF.Relu, bias=bias0[:, 0:1])

                # -- DVE: z1(i) halves --
                z1h = []
                for half in range(2):
                    z_t = z1_pool.tile([128, 16, COLS], dt.bfloat16,
                                       name=f"z1{half}", tag="z1")
                    nc.vector.tensor_tensor(
                        z_t[:], xrh[i][half][:],
                        h1[:].unsqueeze(1).broadcast_to([128, 16, COLS]),
                        ALU.mult)
                    z1h.append(z_t)

                if i == 0 and n_tiles > 1:
                    # primer: fill the pipeline-warmup PE gap (L1(0) waits
                    # on z1(0)) with tile 1's L0 chains (z0(1): preamble).
                    emit_L0(1)

            if i >= 1:
                # -- PE: L2(i-1) --
                c = i - 1
                ps2 = {m: psum_pool.tile([128, COLS], dt.float32,
                                         name=f"ps2{m}", tag="ps")
                       for m in (0, 1)}
                for m in (0, 1):
                    for half in range(2):
                        for g in range(16):
                            nc.tensor.matmul(
                                ps2[m][:],
                                lw2[:, m, half * 16 + g, :],
                                z2t[c][half][:, g, :],
                                start=(half == 0 and g == 0),
                                stop=(half == 1 and g == 15))
                r2a = r_pool.tile([128, COLS], dt.bfloat16, name="r2a", tag="r")
                nc.scalar.activation(r2a[:], ps2[0][:], AF.Relu, bias=bias2[:, 0:1])
                r2b = r_pool.tile([128, COLS], dt.bfloat16, name="r2b", tag="r")
                nc.scalar.activation(r2b[:], ps2[1][:], AF.Relu, bias=bias2[:, 1:2])

            # -- Pool: z0(i+1) (deps: tile i+1 DMAs only) --
            if i + 1 < n_tiles and z0t[i + 1] is None:
                emit_z0(i + 1)

            if i < n_tiles:
                # -- PE: L1(i); m=1 (h-half) first --
                ps1 = {m: psum_pool.tile([128, COLS], dt.float32,
                                         name=f"ps1{m}", tag="ps")
                       for m in (1, 0)}
                for m in (1, 0):
                    for half in range(2):
                        for g in range(16):
                            nc.tensor.matmul(
                                ps1[m][:],
                                lw1[:, m, half * 16 + g, :],
                                z1h[half][:, g, :],
                                start=(half == 0 and g == 0),
                                stop=(half == 1 and g == 15))
                h2 = h_pool.tile([128, COLS], dt.bfloat16, name="h2", tag="h")
                nc.scalar.activation(h2[:], ps1[1][:], AF.Relu, bias=bias1[:, 1:2])
                r1 = r_pool.tile([128, COLS], dt.bfloat16, name="r1", tag="r")
                nc.scalar.activation(r1[:], ps1[0][:], AF.Relu, bias=bias1[:, 0:1])

                # -- DVE: z2(i) halves --
                z2h = []
                for half in range(2):
                    z_t = z2_pool.tile([128, 16, COLS], dt.bfloat16,
                                       name=f"z2{half}", tag="z2")
                    nc.vector.tensor_tensor(
                        z_t[:], xrh[i][half][:],
                        h2[:].unsqueeze(1).broadcast_to([128, 16, COLS]),
                        ALU.mult)
                    z2h.append(z_t)
                z2t[i] = z2h

                # -- DVE: pooling reduces --
                reduce_into(P0, i, r0)
                if i >= 1:
                    reduce_into(P2a, i - 1, r2a)
                    reduce_into(P2b, i - 1, r2b)
                reduce_into(P1, i, r1)
                if i == n_tiles - 1:
                    # P0/P1 are complete now; overlap their write-out with
                    # the final L2 tile
                    nc.sync.dma_start(pout.ap()[0], P0[:])
                    nc.sync.dma_start(pout.ap()[1], P1[:])
            else:
                reduce_into(P2a, i - 1, r2a)
                reduce_into(P2b, i - 1, r2b)

        # ---- ship remaining accumulators; tiny FC happens on host ----
        nc.sync.dma_start(pout.ap()[2], P2a[:])
        nc.sync.dma_start(pout.ap()[3], P2b[:])

    nc.compile()
    return nc


def _pair_indices():
    """Unordered-pair enumeration for the symmetric L0 contraction:
    32 diagonal pairs first, then the 496 f1<f2 pairs (total 528)."""
    ia = [f for f in range(F)]
    ib = [f for f in range(F)]
    for f1 in range(F):
        for f2 in range(f1 + 1, F):
            ia.append(f1)
            ib.append(f2)
    return np.asarray(ia, np.int64), np.asarray(ib, np.int64)


def _prep_inputs(x, w0, b0, w1, b1, w2, b2, fc_w, fc_b):
    bf16 = ml_dtypes.bfloat16
    xb = np.asarray(x, dtype=np.float32).astype(bf16)

    def wprep(w, G):
        # w [O, c] with c = 128*g + p  ->  [p, (g, o)] contiguous per p
        wt = np.asarray(w, np.float32).T.reshape(G, 128, O)  # [g, p, o]
        return np.ascontiguousarray(
            wt.transpose(1, 0, 2).reshape(128, G * O)).astype(bf16)

    def wprep_m(w, G):
        # -> [p, (m, g, o')]: out-half-major so each m-chain DMAs separately
        wt = np.asarray(w, np.float32).T.reshape(G, 128, 2, 128)  # [g,p,m,o']
        return np.ascontiguousarray(
            wt.transpose(1, 2, 0, 3).reshape(128, G * O)).astype(bf16)

    # L0 symmetrization: channel c = 32*f1 + f2, z0[c] = x[f1]*x[f2] is
    # symmetric -> fold to 528 unordered pairs (pad to 640 = 5*128).
    ia, ib = _pair_indices()
    w0f = np.asarray(w0, np.float32).reshape(O, F, F)
    w0s = np.zeros((O, 640), np.float32)
    npair = len(ia)  # 528
    w0s[:, :npair] = w0f[:, ia, ib]
    off = ia != ib
    w0s[:, :npair][:, off] += w0f[:, ib[off], ia[off]]
    w0t = wprep(w0s, 5)
    w1t = wprep_m(w1, 32)
    w2t = wprep_m(w2, 32)
    common = {
        "w0t": w0t, "w1t": w1t, "w2t": w2t,
        "b0": np.ascontiguousarray(np.asarray(b0, np.float32)),
        "b1": np.ascontiguousarray(np.asarray(b1, np.float32)),
        "b2": np.ascontiguousarray(np.asarray(b2, np.float32)),
    }
    # pad pair index lists to 640 (padded channels have zero weight)
    iap = np.zeros(640, np.int64); iap[:len(ia)] = ia
    ibp = np.zeros(640, np.int64); ibp[:len(ib)] = ib
    in_maps = []
    for c in range(N_CORES):
        m = dict(common)
        xc = xb[c * BC:(c + 1) * BC]                     # [BC, F, E]
        # xtile[t, f, (b e)] = x[8t+b, f, e]
        xt = np.ascontiguousarray(
            xc.reshape(NT, NB, F, E).transpose(0, 2, 1, 3).reshape(NT, F, COLS))
        m["xtile"] = xt
        # xpa/xpb[t, p, (g c)] = xtile[t, idx[128g+p], c]  (pair operands)
        for nm, idx in (("xpa", iap), ("xpb", ibp)):
            g = xt[:, idx, :]                            # [NT, 640, COLS]
            m[nm] = np.ascontiguousarray(
                g.reshape(NT, 5, 128, COLS).transpose(0, 2, 1, 3)
                 .reshape(NT, 128, 5 * COLS))
        in_maps.append(m)
    return in_maps


def kernel(x, w0, b0, w1, b1, w2, b2, fc_w, fc_b, **kw):
    from concourse.bass_utils import run_bass_kernel_spmd

    if "nc" not in _CACHE:
        _CACHE["nc"] = _build()
    nc = _CACHE["nc"]
    in_maps = _prep_inputs(x, w0, b0, w1, b1, w2, b2, fc_w, fc_b)
    res = run_bass_kernel_spmd(nc, in_maps, list(range(N_CORES)))
    fcw = np.asarray(fc_w, np.float32).reshape(4, 128)
    ys = []
    for c in range(N_CORES):
        p = res.results[c]["pout"]  # [4, 128, BC]
        ys.append(np.einsum('cp,cpb->b', fcw, p.astype(np.float32)))
    out = np.concatenate(ys).reshape(B, 1).astype(np.float32)
    out = out + np.asarray(fc_b, np.float32).reshape(1, 1)
    return out


# revision 44
# speedup vs baseline: 1.2915x; 1.0496x over previous
"""CIN (xDeepFM CompressedInteractionNetwork) forward on 8 TRN2 NeuronCores.

Strategy (pure data parallelism, hardcoded from the problem spec):
  - batch 4096 -> 512 per core; 64 tiles of 8 batch elements; matmul free
    dim = 512 columns = (8 batch x 64 embed).
  - layer l: out[o, col] = relu( sum_c W[o,c] * z[c, col] + b[o] ) where
    z[f*Hin+j, col] = x0[f, col] * h[j, col].  z is materialized as bf16
    tensor_tensor multiplies (x0 rows partition-broadcast via DMA, h
    broadcast along a stride-0 free dim).
  - matmuls in bf16 (full PE rate), fp32 PSUM; ScalarE applies bias+relu
    out of PSUM; DVE pools (sum over embed); final FC on host.

Perf structure:
  - software-pipelined PE stream: per iteration i the tensor engine runs
    L0(i), L2(i-1), L1(i); each layer's h-producing chain (m=1) first.
    All PE dependencies are then ~a full layer old -> no 8us z-build
    stalls (the v1 baseline lost 1.0ms/core to 128 such gaps and sat at
    the 1.2GHz p-state; a dense stream sustains 2.4GHz).
  - L0 exploits z0 = x (x) x symmetry: the 1024-channel contraction
    folds to 528 unordered pairs (symmetrized W0), padded to 640=5x128
    -> 10 instead of 16 matmuls per tile. Pair operands are host-
    gathered arrays, plain per-partition-contiguous DMA loads.
  - z0-build on the Pool engine (gpsimd), z1/z2 builds + pooling reduces
    on DVE (r tiles bf16 for 2x DVE reduce rate).
  - host pre-lays x out per-tile ([tile, f, b, e]) so every partition-
    broadcast DMA reads a contiguous 16KB block -> 16KB descriptors
    instead of 1KB (v1: 16 DMA engines x 1.65ms busy; now ~0.6ms).
  - warmup: tile-1 L0 primed into iteration 0, startup DMAs ordered by
    first-use time; first matmul issues at ~10us.
"""

import sys

sys.path.insert(0, "/opt/trn_rl_repo")

import numpy as np
import ml_dtypes
from contextlib import ExitStack

N_CORES = 8
B = 4096
F = 32
E = 64
BC = B // N_CORES  # 512 batch elements per core
NB = 8             # batch elements per tile
COLS = NB * E      # 512 matmul columns per tile
NT = BC // NB      # 64 tiles per core
O = 256            # conv out channels per layer

_CACHE = {}


def _build(n_tiles=NT):
    import concourse.bass as bass  # noqa: F401
    import concourse.mybir as mybir
    import concourse.tile as tile
    from concourse import bacc

    dt = mybir.dt
    AF = mybir.ActivationFunctionType
    ALU = mybir.AluOpType
    AX = mybir.AxisListType

    nc = bacc.Bacc("TRN2", target_bir_lowering=False, debug=False,
                   num_devices=N_CORES)

    # x pre-laid out per tile: xtile[t, f, (b e)] so broadcast DMAs read
    # contiguous blocks. L0 uses the z0 symmetry (x (x) x): 1024 channels
    # collapse to 528 unordered pairs, padded to 640 = 5x128; xpa/xpb are
    # host-gathered pair operands in [t, p, (g c)] per-partition layout.
    xtile = nc.declare_dram_parameter("xtile", [n_tiles, F, COLS],
                                      dt.bfloat16, isOutput=False)
    xpa = nc.declare_dram_parameter("xpa", [n_tiles, 128, 5 * COLS],
                                    dt.bfloat16, isOutput=False)
    xpb = nc.declare_dram_parameter("xpb", [n_tiles, 128, 5 * COLS],
                                    dt.bfloat16, isOutput=False)
    # weights host-prepped to [p, g, o] (per-partition contiguous blocks);
    # w0t is the symmetrized+padded L0 weight (640 channels).
    w0t = nc.declare_dram_parameter("w0t", [128, 5 * O], dt.bfloat16, isOutput=False)
    w1t = nc.declare_dram_parameter("w1t", [128, 32 * O], dt.bfloat16, isOutput=False)
    # L2 runs fp8e4 DoubleRow (2 contraction groups/matmul, 0.5 cyc/row)
    w2t = nc.declare_dram_parameter("w2t", [128, 32 * O], dt.float8e4, isOutput=False)
    b0 = nc.declare_dram_parameter("b0", [O], dt.float32, isOutput=False)
    b1 = nc.declare_dram_parameter("b1", [O], dt.float32, isOutput=False)
    b2 = nc.declare_dram_parameter("b2", [O], dt.float32, isOutput=False)
    pout = nc.declare_dram_parameter("pout", [4, 128, n_tiles * NB],
                                     dt.float32, isOutput=True)

    with ExitStack() as ctx:
        tc = ctx.enter_context(tile.TileContext(nc))
        const = ctx.enter_context(tc.tile_pool(name="const", bufs=1))

        # ---- persistent weights / biases (DMAs interleaved below so tile-0
        #      prefetch isn't stuck behind 4.5MB of weights) ----
        lw0 = const.tile([128, 5, O], dt.bfloat16)       # w chunk [c=128g+p]
        # lw1/lw2 split by out-half m so each chain's weights can be DMA'd
        # separately in first-use order: [p, m, g, o']
        lw1 = const.tile([128, 2, 32, 128], dt.bfloat16)
        lw2 = const.tile([128, 2, 32, 128], dt.float8e4)
        bias0 = const.tile([128, 2], dt.float32)
        bias1 = const.tile([128, 2], dt.float32)
        bias2 = const.tile([128, 2], dt.float32)

        # pooled accumulators [o_chunk 128, batch 512]
        P0 = const.tile([128, n_tiles * NB], dt.float32)
        P1 = const.tile([128, n_tiles * NB], dt.float32)
        P2a = const.tile([128, n_tiles * NB], dt.float32)
        P2b = const.tile([128, n_tiles * NB], dt.float32)

        # ---- rotating pools ----
        xr_pool = ctx.enter_context(tc.tile_pool(name="xr", bufs=2))
        za_pool = ctx.enter_context(tc.tile_pool(name="za", bufs=2))
        zb_pool = ctx.enter_context(tc.tile_pool(name="zb", bufs=2))
        z0_pool = ctx.enter_context(tc.tile_pool(name="z0", bufs=2))
        z1_pool = ctx.enter_context(tc.tile_pool(name="z1", bufs=2))
        z2bf_pool = ctx.enter_context(tc.tile_pool(name="z2bf", bufs=2))
        z2_pool = ctx.enter_context(tc.tile_pool(name="z2", bufs=4))
        h_pool = ctx.enter_context(tc.tile_pool(name="h", bufs=2))
        r_pool = ctx.enter_context(tc.tile_pool(name="r", bufs=4))
        psum_pool = ctx.enter_context(tc.tile_pool(name="ps", bufs=6, space="PSUM"))

        # per-tile state carried across pipeline stages
        xrh = [None] * n_tiles   # [2 x tile [128,16,COLS]]
        z0t = [None] * n_tiles
        z2t = [None] * n_tiles
        za_t = [None] * n_tiles
        zb_t = [None] * n_tiles

        def emit_dma_pair(t):
            """Prefetch tile t's L0 pair operands (gate z0(t) -> L0(t))."""
            za = za_pool.tile([128, 5, COLS], dt.bfloat16)
            nc.sync.dma_start(za[:].rearrange("p g c -> p (g c)"), xpa.ap()[t])
            za_t[t] = za
            zb = zb_pool.tile([128, 5, COLS], dt.bfloat16)
            nc.sync.dma_start(zb[:].rearrange("p g c -> p (g c)"), xpb.ap()[t])
            zb_t[t] = zb

        def emit_dma_xr(t):
            """Prefetch tile t's replicated x rows (feed z1/z2 builds)."""
            halves = []
            for half in range(2):
                xr_t = xr_pool.tile([128, 16, COLS], dt.bfloat16,
                                    name=f"xr{half}", tag="xr")
                src = xtile.ap()[t, half * 16:(half + 1) * 16, :] \
                    .unsqueeze(0).broadcast_to([128, 16, COLS])
                nc.sync.dma_start(xr_t[:], src)
                halves.append(xr_t)
            xrh[t] = halves

        def emit_dma(t):
            emit_dma_pair(t)
            emit_dma_xr(t)

        def emit_z0(t, eng=None):
            """z0(t) on the Pool engine (keeps DVE free for z1/z2)."""
            z0 = z0_pool.tile([128, 5, COLS], dt.bfloat16)
            (eng or nc.gpsimd).tensor_tensor(
                z0[:], za_t[t][:], zb_t[t][:], ALU.mult)
            z0t[t] = z0

        def reduce_into(P, t, r_t):
            nc.vector.tensor_reduce(
                P[:, t * NB:(t + 1) * NB],
                r_t[:].rearrange("p (b e) -> p b e", e=E), AX.X, ALU.add)

        # ---- preamble: startup DMAs ordered by first-use time ----
        nc.sync.dma_start(lw0[:], w0t.ap().rearrange("p (g o) -> p g o", o=O))
        nc.sync.dma_start(bias0[:], b0.ap().rearrange("(m p) -> p m", p=128))
        emit_dma_pair(0)
        emit_z0(0, eng=nc.vector)   # DVE: fast; gates very first matmul
        if n_tiles > 1:
            emit_dma_pair(1)
            emit_z0(1, eng=nc.vector)  # gates the iter-0 primer L0(1)
        emit_dma_xr(0)
        w1v = w1t.ap().rearrange("p (m x) -> p m x", m=2)
        nc.sync.dma_start(lw1[:, 1].rearrange("p g o -> p (g o)"), w1v[:, 1])
        nc.sync.dma_start(bias1[:], b1.ap().rearrange("(m p) -> p m", p=128))
        nc.sync.dma_start(lw1[:, 0].rearrange("p g o -> p (g o)"), w1v[:, 0])
        if n_tiles > 1:
            emit_dma_xr(1)
        w2v = w2t.ap().rearrange("p (m x) -> p m x", m=2)
        nc.sync.dma_start(lw2[:, 0].rearrange("p g o -> p (g o)"), w2v[:, 0])
        nc.sync.dma_start(bias2[:], b2.ap().rearrange("(m p) -> p m", p=128))
        nc.sync.dma_start(lw2[:, 1].rearrange("p g o -> p (g o)"), w2v[:, 1])

        ps0_of = {}

        def emit_L0(t):
            """PE chains of L0(t); m=1 (h-half) first."""
            ps0 = {m: psum_pool.tile([128, COLS], dt.float32,
                                     name=f"ps0{m}", tag="ps")
                   for m in (1, 0)}
            for m in (1, 0):
                for g in range(5):
                    nc.tensor.matmul(
                        ps0[m][:], lw0[:, g, m * 128:(m + 1) * 128],
                        z0t[t][:, g, :], start=(g == 0), stop=(g == 4))
            ps0_of[t] = ps0

        for i in range(n_tiles + 2):
            if i + 1 < n_tiles and za_t[i + 1] is None:
                emit_dma(i + 1)

            if i < n_tiles:
                if i not in ps0_of:
                    emit_L0(i)
                ps0 = ps0_of.pop(i)
                h1 = h_pool.tile([128, COLS], dt.bfloat16, name="h1", tag="h")
                nc.scalar.activation(h1[:], ps0[1][:], AF.Relu, bias=bias0[:, 1:2])
                r0 = r_pool.tile([128, COLS], dt.bfloat16, name="r0", tag="r")
                nc.scalar.activation(r0[:], ps0[0][:], AF.Relu, bias=bias0[:, 0:1])

                # -- DVE: z1(i) halves --
                z1h = []
                for half in range(2):
                    z_t = z1_pool.tile([128, 16, COLS], dt.bfloat16,
                                       name=f"z1{half}", tag="z1")
                    nc.vector.tensor_tensor(
                        z_t[:], xrh[i][half][:],
                        h1[:].unsqueeze(1).broadcast_to([128, 16, COLS]),
                        ALU.mult)
                    z1h.append(z_t)

                if i == 0 and n_tiles > 1:
                    # primer: fill the pipeline-warmup PE gap (L1(0) waits
                    # on z1(0)) with tile 1's L0 chains (z0(1): preamble).
                    emit_L0(1)

            if i >= 2:
                # -- PE: L2(i-2) fp8 DoubleRow (z2f converted ~an iter ago) --
                c = i - 2
                ps2 = {m: psum_pool.tile([128, COLS], dt.float32,
                                         name=f"ps2{m}", tag="ps")
                       for m in (0, 1)}
                for m in (0, 1):
                    for half in range(2):
                        for g2 in range(8):
                            nc.tensor.matmul(
                                ps2[m][:],
                                lw2[:, m, half * 16 + 2 * g2:
                                    half * 16 + 2 * g2 + 2, :],
                                z2t[c][half][:, 2 * g2:2 * g2 + 2, :],
                                start=(half == 0 and g2 == 0),
                                stop=(half == 1 and g2 == 7),
                                perf_mode=mybir.MatmulPerfMode.DoubleRow)
                r2a = r_pool.tile([128, COLS], dt.bfloat16, name="r2a", tag="r")
                nc.scalar.activation(r2a[:], ps2[0][:], AF.Relu, bias=bias2[:, 0:1])
                r2b = r_pool.tile([128, COLS], dt.bfloat16, name="r2b", tag="r")
                nc.scalar.activation(r2b[:], ps2[1][:], AF.Relu, bias=bias2[:, 1:2])
                reduce_into(P2a, c, r2a)
                reduce_into(P2b, c, r2b)

            # -- Pool: z0(i+1) (deps: tile i+1 DMAs only) --
            if i + 1 < n_tiles and z0t[i + 1] is None:
                emit_z0(i + 1)

            if i < n_tiles:
                # -- PE: L1(i); m=1 (h-half) first --
                ps1 = {m: psum_pool.tile([128, COLS], dt.float32,
                                         name=f"ps1{m}", tag="ps")
                       for m in (1, 0)}
                for m in (1, 0):
                    for half in range(2):
                        for g in range(16):
                            nc.tensor.matmul(
                                ps1[m][:],
                                lw1[:, m, half * 16 + g, :],
                                z1h[half][:, g, :],
                                start=(half == 0 and g == 0),
                                stop=(half == 1 and g == 15))
                h2 = h_pool.tile([128, COLS], dt.bfloat16, name="h2", tag="h")
                nc.scalar.activation(h2[:], ps1[1][:], AF.Relu, bias=bias1[:, 1:2])
                r1 = r_pool.tile([128, COLS], dt.bfloat16, name="r1", tag="r")
                nc.scalar.activation(r1[:], ps1[0][:], AF.Relu, bias=bias1[:, 0:1])

                # -- DVE: z2(i) halves in bf16 (2x mode), then cast to fp8
                #    on the otherwise-idle scalar engine --
                z2h = []
                for half in range(2):
                    zbf = z2bf_pool.tile([128, 16, COLS], dt.bfloat16,
                                         name=f"z2bf{half}", tag="z2bf")
                    nc.vector.tensor_tensor(
                        zbf[:], xrh[i][half][:],
                        h2[:].unsqueeze(1).broadcast_to([128, 16, COLS]),
                        ALU.mult)
                    zf = z2_pool.tile([128, 16, COLS], dt.float8e4,
                                      name=f"z2f{half}", tag="z2")
                    nc.scalar.activation(zf[:], zbf[:], AF.Copy)
                    z2h.append(zf)
                z2t[i] = z2h

                # -- DVE: pooling reduces --
                reduce_into(P0, i, r0)
                reduce_into(P1, i, r1)
                if i == n_tiles - 1:
                    # P0/P1 are complete now; overlap their write-out with
                    # the final L2 tiles
                    nc.sync.dma_start(pout.ap()[0], P0[:])
                    nc.sync.dma_start(pout.ap()[1], P1[:])

        # ---- ship remaining accumulators; tiny FC happens on host ----
        nc.sync.dma_start(pout.ap()[2], P2a[:])
        nc.sync.dma_start(pout.ap()[3], P2b[:])

    nc.compile()
    return nc


def _pair_indices():
    """Unordered-pair enumeration for the symmetric L0 contraction:
    32 diagonal pairs first, then the 496 f1<f2 pairs (total 528)."""
    ia = [f for f in range(F)]
    ib = [f for f in range(F)]
    for f1 in range(F):
        for f2 in range(f1 + 1, F):
            ia.append(f1)
            ib.append(f2)
    return np.asarray(ia, np.int64), np.asarray(ib, np.int64)


def _prep_inputs(x, w0, b0, w1, b1, w2, b2, fc_w, fc_b):
    bf16 = ml_dtypes.bfloat16
    xb = np.asarray(x, dtype=np.float32).astype(bf16)

    def wprep(w, G):
        # w [O, c] with c = 128*g + p  ->  [p, (g, o)] contiguous per p
        wt = np.asarray(w, np.float32).T.reshape(G, 128, O)  # [g, p, o]
        return np.ascontiguousarray(
            wt.transpose(1, 0, 2).reshape(128, G * O)).astype(bf16)

    def wprep_m(w, G, dtype=None):
        # -> [p, (m, g, o')]: out-half-major so each m-chain DMAs separately
        wt = np.asarray(w, np.float32).T.reshape(G, 128, 2, 128)  # [g,p,m,o']
        return np.ascontiguousarray(
            wt.transpose(1, 2, 0, 3).reshape(128, G * O)).astype(dtype or bf16)

    # L0 symmetrization: channel c = 32*f1 + f2, z0[c] = x[f1]*x[f2] is
    # symmetric -> fold to 528 unordered pairs (pad to 640 = 5*128).
    ia, ib = _pair_indices()
    w0f = np.asarray(w0, np.float32).reshape(O, F, F)
    w0s = np.zeros((O, 640), np.float32)
    npair = len(ia)  # 528
    w0s[:, :npair] = w0f[:, ia, ib]
    off = ia != ib
    w0s[:, :npair][:, off] += w0f[:, ib[off], ia[off]]
    w0t = wprep(w0s, 5)
    w1t = wprep_m(w1, 32)
    w2t = wprep_m(w2, 32, dtype=ml_dtypes.float8_e4m3)
    common = {
        "w0t": w0t, "w1t": w1t, "w2t": w2t,
        "b0": np.ascontiguousarray(np.asarray(b0, np.float32)),
        "b1": np.ascontiguousarray(np.asarray(b1, np.float32)),
        "b2": np.ascontiguousarray(np.asarray(b2, np.float32)),
    }
    # pad pair index lists to 640 (padded channels have zero weight)
    iap = np.zeros(640, np.int64); iap[:len(ia)] = ia
    ibp = np.zeros(640, np.int64); ibp[:len(ib)] = ib
    in_maps = []
    for c in range(N_CORES):
        m = dict(common)
        xc = xb[c * BC:(c + 1) * BC]                     # [BC, F, E]
        # xtile[t, f, (b e)] = x[8t+b, f, e]
        xt = np.ascontiguousarray(
            xc.reshape(NT, NB, F, E).transpose(0, 2, 1, 3).reshape(NT, F, COLS))
        m["xtile"] = xt
        # xpa/xpb[t, p, (g c)] = xtile[t, idx[128g+p], c]  (pair operands)
        for nm, idx in (("xpa", iap), ("xpb", ibp)):
            g = xt[:, idx, :]                            # [NT, 640, COLS]
            m[nm] = np.ascontiguousarray(
                g.reshape(NT, 5, 128, COLS).transpose(0, 2, 1, 3)
                 .reshape(NT, 128, 5 * COLS))
        in_maps.append(m)
    return in_maps


def kernel(x, w0, b0, w1, b1, w2, b2, fc_w, fc_b, **kw):
    from concourse.bass_utils import run_bass_kernel_spmd

    if "nc" not in _CACHE:
        _CACHE["nc"] = _build()
    nc = _CACHE["nc"]
    in_maps = _prep_inputs(x, w0, b0, w1, b1, w2, b2, fc_w, fc_b)
    res = run_bass_kernel_spmd(nc, in_maps, list(range(N_CORES)))
    fcw = np.asarray(fc_w, np.float32).reshape(4, 128)
    ys = []
    for c in range(N_CORES):
        p = res.results[c]["pout"]  # [4, 128, BC]
        ys.append(np.einsum('cp,cpb->b', fcw, p.astype(np.float32)))
    out = np.concatenate(ys).reshape(B, 1).astype(np.float32)
    out = out + np.asarray(fc_b, np.float32).reshape(1, 1)
    return out


# revision 45
# speedup vs baseline: 1.4770x; 1.1437x over previous
"""CIN (xDeepFM CompressedInteractionNetwork) forward on 8 TRN2 NeuronCores.

Strategy (pure data parallelism, hardcoded from the problem spec):
  - batch 4096 -> 512 per core; 64 tiles of 8 batch elements; matmul free
    dim = 512 columns = (8 batch x 64 embed).
  - layer l: out[o, col] = relu( sum_c W[o,c] * z[c, col] + b[o] ) where
    z[f*Hin+j, col] = x0[f, col] * h[j, col].  z is materialized as bf16
    tensor_tensor multiplies (x0 rows partition-broadcast via DMA, h
    broadcast along a stride-0 free dim).
  - matmuls in bf16 (full PE rate), fp32 PSUM; ScalarE applies bias+relu
    out of PSUM; DVE pools (sum over embed); final FC on host.

Perf structure:
  - software-pipelined PE stream: per iteration i the tensor engine runs
    L0(i), L2(i-1), L1(i); each layer's h-producing chain (m=1) first.
    All PE dependencies are then ~a full layer old -> no 8us z-build
    stalls (the v1 baseline lost 1.0ms/core to 128 such gaps and sat at
    the 1.2GHz p-state; a dense stream sustains 2.4GHz).
  - L0 exploits z0 = x (x) x symmetry: the 1024-channel contraction
    folds to 528 unordered pairs (symmetrized W0), padded to 640=5x128
    -> 10 instead of 16 matmuls per tile. Pair operands are host-
    gathered arrays, plain per-partition-contiguous DMA loads.
  - z0-build on the Pool engine (gpsimd), z1/z2 builds + pooling reduces
    on DVE (r tiles bf16 for 2x DVE reduce rate).
  - host pre-lays x out per-tile ([tile, f, b, e]) so every partition-
    broadcast DMA reads a contiguous 16KB block -> 16KB descriptors
    instead of 1KB (v1: 16 DMA engines x 1.65ms busy; now ~0.6ms).
  - warmup: tile-1 L0 primed into iteration 0, startup DMAs ordered by
    first-use time; first matmul issues at ~10us.
"""

import sys

sys.path.insert(0, "/opt/trn_rl_repo")

import numpy as np
import ml_dtypes
from contextlib import ExitStack

N_CORES = 8
B = 4096
F = 32
E = 64
BC = B // N_CORES  # 512 batch elements per core
NB = 8             # batch elements per tile
COLS = NB * E      # 512 matmul columns per tile
NT = BC // NB      # 64 tiles per core
O = 256            # conv out channels per layer

_CACHE = {}


def _build(n_tiles=NT):
    import concourse.bass as bass  # noqa: F401
    import concourse.mybir as mybir
    import concourse.tile as tile
    from concourse import bacc

    dt = mybir.dt
    AF = mybir.ActivationFunctionType
    ALU = mybir.AluOpType
    AX = mybir.AxisListType

    nc = bacc.Bacc("TRN2", target_bir_lowering=False, debug=False,
                   num_devices=N_CORES)

    # x pre-laid out per tile: xtile[t, f, (b e)] so broadcast DMAs read
    # contiguous blocks. L0 uses the z0 symmetry (x (x) x): 1024 channels
    # collapse to 528 unordered pairs, padded to 640 = 5x128; xpa/xpb are
    # host-gathered pair operands in [t, p, (g c)] per-partition layout.
    xtile = nc.declare_dram_parameter("xtile", [n_tiles, F, COLS],
                                      dt.bfloat16, isOutput=False)
    xpa = nc.declare_dram_parameter("xpa", [n_tiles, 128, 5 * COLS],
                                    dt.bfloat16, isOutput=False)
    xpb = nc.declare_dram_parameter("xpb", [n_tiles, 128, 5 * COLS],
                                    dt.bfloat16, isOutput=False)
    # weights host-prepped to [p, g, o] (per-partition contiguous blocks);
    # w0t is the symmetrized+padded L0 weight (640 channels).
    w0t = nc.declare_dram_parameter("w0t", [128, 5 * O], dt.bfloat16, isOutput=False)
    w1t = nc.declare_dram_parameter("w1t", [128, 32 * O], dt.bfloat16, isOutput=False)
    w2t = nc.declare_dram_parameter("w2t", [128, 32 * O], dt.bfloat16, isOutput=False)
    b0 = nc.declare_dram_parameter("b0", [O], dt.float32, isOutput=False)
    b1 = nc.declare_dram_parameter("b1", [O], dt.float32, isOutput=False)
    b2 = nc.declare_dram_parameter("b2", [O], dt.float32, isOutput=False)
    pout = nc.declare_dram_parameter("pout", [4, 128, n_tiles * NB],
                                     dt.float32, isOutput=True)

    with ExitStack() as ctx:
        tc = ctx.enter_context(tile.TileContext(nc))
        const = ctx.enter_context(tc.tile_pool(name="const", bufs=1))

        # ---- persistent weights / biases (DMAs interleaved below so tile-0
        #      prefetch isn't stuck behind 4.5MB of weights) ----
        lw0 = const.tile([128, 5, O], dt.bfloat16)       # w chunk [c=128g+p]
        # lw1/lw2 split by out-half m so each chain's weights can be DMA'd
        # separately in first-use order: [p, m, g, o']
        lw1 = const.tile([128, 2, 32, 128], dt.bfloat16)
        lw2 = const.tile([128, 2, 32, 128], dt.bfloat16)
        bias0 = const.tile([128, 2], dt.float32)
        bias1 = const.tile([128, 2], dt.float32)
        bias2 = const.tile([128, 2], dt.float32)

        # pooled accumulators [o_chunk 128, batch 512]
        P0 = const.tile([128, n_tiles * NB], dt.float32)
        P1 = const.tile([128, n_tiles * NB], dt.float32)
        P2a = const.tile([128, n_tiles * NB], dt.float32)
        P2b = const.tile([128, n_tiles * NB], dt.float32)

        # ---- rotating pools ----
        xr_pool = ctx.enter_context(tc.tile_pool(name="xr", bufs=3))
        za_pool = ctx.enter_context(tc.tile_pool(name="za", bufs=2))
        zb_pool = ctx.enter_context(tc.tile_pool(name="zb", bufs=2))
        z0_pool = ctx.enter_context(tc.tile_pool(name="z0", bufs=2))
        z1_pool = ctx.enter_context(tc.tile_pool(name="z1", bufs=2))
        z2_pool = ctx.enter_context(tc.tile_pool(name="z2", bufs=2))
        h_pool = ctx.enter_context(tc.tile_pool(name="h", bufs=3))
        r_pool = ctx.enter_context(tc.tile_pool(name="r", bufs=4))
        psum_pool = ctx.enter_context(tc.tile_pool(name="ps", bufs=6, space="PSUM"))

        # per-tile state carried across pipeline stages
        xrh = [None] * n_tiles   # [2 x tile [128,16,COLS]]
        z0t = [None] * n_tiles
        z2t = [None] * n_tiles
        za_t = [None] * n_tiles
        zb_t = [None] * n_tiles

        def emit_dma_pair(t):
            """Prefetch tile t's L0 pair operands (gate z0(t) -> L0(t))."""
            za = za_pool.tile([128, 5, COLS], dt.bfloat16)
            nc.sync.dma_start(za[:].rearrange("p g c -> p (g c)"), xpa.ap()[t])
            za_t[t] = za
            zb = zb_pool.tile([128, 5, COLS], dt.bfloat16)
            nc.sync.dma_start(zb[:].rearrange("p g c -> p (g c)"), xpb.ap()[t])
            zb_t[t] = zb

        def emit_dma_xr(t):
            """Prefetch tile t's replicated x rows (feed z1/z2 builds)."""
            halves = []
            for half in range(2):
                xr_t = xr_pool.tile([128, 16, COLS], dt.bfloat16,
                                    name=f"xr{half}", tag="xr")
                src = xtile.ap()[t, half * 16:(half + 1) * 16, :] \
                    .unsqueeze(0).broadcast_to([128, 16, COLS])
                nc.sync.dma_start(xr_t[:], src)
                halves.append(xr_t)
            xrh[t] = halves

        def emit_dma(t):
            emit_dma_pair(t)
            emit_dma_xr(t)

        def emit_z0(t, eng=None):
            """z0(t) on the Pool engine (keeps DVE free for z1/z2)."""
            z0 = z0_pool.tile([128, 5, COLS], dt.bfloat16)
            (eng or nc.gpsimd).tensor_tensor(
                z0[:], za_t[t][:], zb_t[t][:], ALU.mult)
            z0t[t] = z0

        def reduce_into(P, t, r_t):
            nc.vector.tensor_reduce(
                P[:, t * NB:(t + 1) * NB],
                r_t[:].rearrange("p (b e) -> p b e", e=E), AX.X, ALU.add)

        # ---- preamble: startup DMAs ordered by first-use time ----
        nc.sync.dma_start(lw0[:], w0t.ap().rearrange("p (g o) -> p g o", o=O))
        nc.sync.dma_start(bias0[:], b0.ap().rearrange("(m p) -> p m", p=128))
        emit_dma_pair(0)
        emit_z0(0, eng=nc.vector)   # DVE: fast; gates very first matmul
        if n_tiles > 1:
            emit_dma_pair(1)
            emit_z0(1, eng=nc.vector)  # gates the iter-0 primer L0(1)
        emit_dma_xr(0)
        w1v = w1t.ap().rearrange("p (m x) -> p m x", m=2)
        nc.sync.dma_start(lw1[:, 1].rearrange("p g o -> p (g o)"), w1v[:, 1])
        nc.sync.dma_start(bias1[:], b1.ap().rearrange("(m p) -> p m", p=128))
        nc.sync.dma_start(lw1[:, 0].rearrange("p g o -> p (g o)"), w1v[:, 0])
        if n_tiles > 1:
            emit_dma_xr(1)
        w2v = w2t.ap().rearrange("p (m x) -> p m x", m=2)
        nc.sync.dma_start(lw2[:, 0].rearrange("p g o -> p (g o)"), w2v[:, 0])
        nc.sync.dma_start(bias2[:], b2.ap().rearrange("(m p) -> p m", p=128))
        nc.sync.dma_start(lw2[:, 1].rearrange("p g o -> p (g o)"), w2v[:, 1])

        ps0_of = {}

        def emit_L0(t):
            """PE chains of L0(t); m=1 (h-half) first."""
            ps0 = {m: psum_pool.tile([128, COLS], dt.float32,
                                     name=f"ps0{m}", tag="ps")
                   for m in (1, 0)}
            for m in (1, 0):
                for g in range(5):
                    nc.tensor.matmul(
                        ps0[m][:], lw0[:, g, m * 128:(m + 1) * 128],
                        z0t[t][:, g, :], start=(g == 0), stop=(g == 4))
            ps0_of[t] = ps0

        for i in range(n_tiles + 1):
            if i + 1 < n_tiles and za_t[i + 1] is None:
                emit_dma(i + 1)

            if i < n_tiles:
                if i not in ps0_of:
                    emit_L0(i)
                ps0 = ps0_of.pop(i)
                h1 = h_pool.tile([128, COLS], dt.bfloat16, name="h1", tag="h")
                nc.scalar.activation(h1[:], ps0[1][:], AF.Relu, bias=bias0[:, 1:2])
                r0 = r_pool.tile([128, COLS], dt.bfloat16, name="r0", tag="r")
                nc.scalar.activation(r0[:], ps0[0][:], AF.Relu, bias=bias0[:, 0:1])

                # -- DVE: z1(i) halves --
                z1h = []
                for half in range(2):
                    z_t = z1_pool.tile([128, 16, COLS], dt.bfloat16,
                                       name=f"z1{half}", tag="z1")
                    nc.vector.tensor_tensor(
                        z_t[:], xrh[i][half][:],
                        h1[:].unsqueeze(1).broadcast_to([128, 16, COLS]),
                        ALU.mult)
                    z1h.append(z_t)

                if i == 0 and n_tiles > 1:
                    # primer: fill the pipeline-warmup PE gap (L1(0) waits
                    # on z1(0)) with tile 1's L0 chains (z0(1): preamble).
                    emit_L0(1)

            if i >= 1:
                # -- PE: L2(i-1) --
                c = i - 1
                ps2 = {m: psum_pool.tile([128, COLS], dt.float32,
                                         name=f"ps2{m}", tag="ps")
                       for m in (0, 1)}
                for m in (0, 1):
                    for half in range(2):
                        for g in range(16):
                            nc.tensor.matmul(
                                ps2[m][:],
                                lw2[:, m, half * 16 + g, :],
                                z2t[c][half][:, g, :],
                                start=(half == 0 and g == 0),
                                stop=(half == 1 and g == 15))
                r2a = r_pool.tile([128, COLS], dt.bfloat16, name="r2a", tag="r")
                nc.scalar.activation(r2a[:], ps2[0][:], AF.Relu, bias=bias2[:, 0:1])
                r2b = r_pool.tile([128, COLS], dt.bfloat16, name="r2b", tag="r")
                nc.scalar.activation(r2b[:], ps2[1][:], AF.Relu, bias=bias2[:, 1:2])

            # -- Pool: z0(i+1) (deps: tile i+1 DMAs only) --
            if i + 1 < n_tiles and z0t[i + 1] is None:
                emit_z0(i + 1)

            if i < n_tiles:
                # -- PE: L1(i); m=1 (h-half) first --
                ps1 = {m: psum_pool.tile([128, COLS], dt.float32,
                                         name=f"ps1{m}", tag="ps")
                       for m in (1, 0)}
                for m in (1, 0):
                    for half in range(2):
                        for g in range(16):
                            nc.tensor.matmul(
                                ps1[m][:],
                                lw1[:, m, half * 16 + g, :],
                                z1h[half][:, g, :],
                                start=(half == 0 and g == 0),
                                stop=(half == 1 and g == 15))
                h2 = h_pool.tile([128, COLS], dt.bfloat16, name="h2", tag="h")
                nc.scalar.activation(h2[:], ps1[1][:], AF.Relu, bias=bias1[:, 1:2])
                r1 = r_pool.tile([128, COLS], dt.bfloat16, name="r1", tag="r")
                nc.scalar.activation(r1[:], ps1[0][:], AF.Relu, bias=bias1[:, 0:1])

                # -- DVE: z2(i) halves --
                z2h = []
                for half in range(2):
                    z_t = z2_pool.tile([128, 16, COLS], dt.bfloat16,
                                       name=f"z2{half}", tag="z2")
                    nc.vector.tensor_tensor(
                        z_t[:], xrh[i][half][:],
                        h2[:].unsqueeze(1).broadcast_to([128, 16, COLS]),
                        ALU.mult)
                    z2h.append(z_t)
                z2t[i] = z2h

                # -- DVE: pooling reduces --
                reduce_into(P0, i, r0)
                if i >= 1:
                    reduce_into(P2a, i - 1, r2a)
                    reduce_into(P2b, i - 1, r2b)
                reduce_into(P1, i, r1)
                if i == n_tiles - 1:
                    # P0/P1 are complete now; overlap their write-out with
                    # the final L2 tile
                    nc.sync.dma_start(pout.ap()[0], P0[:])
                    nc.sync.dma_start(pout.ap()[1], P1[:])
            else:
                reduce_into(P2a, i - 1, r2a)
                reduce_into(P2b, i - 1, r2b)

        # ---- ship remaining accumulators; tiny FC happens on host ----
        nc.sync.dma_start(pout.ap()[2], P2a[:])
        nc.sync.dma_start(pout.ap()[3], P2b[:])

    nc.compile()
    return nc


def _pair_indices():
    """Unordered-pair enumeration for the symmetric L0 contraction:
    32 diagonal pairs first, then the 496 f1<f2 pairs (total 528)."""
    ia = [f for f in range(F)]
    ib = [f for f in range(F)]
    for f1 in range(F):
        for f2 in range(f1 + 1, F):
            ia.append(f1)
            ib.append(f2)
    return np.asarray(ia, np.int64), np.asarray(ib, np.int64)


def _prep_inputs(x, w0, b0, w1, b1, w2, b2, fc_w, fc_b):
    bf16 = ml_dtypes.bfloat16
    xb = np.asarray(x, dtype=np.float32).astype(bf16)

    def wprep(w, G):
        # w [O, c] with c = 128*g + p  ->  [p, (g, o)] contiguous per p
        wt = np.asarray(w, np.float32).T.reshape(G, 128, O)  # [g, p, o]
        return np.ascontiguousarray(
            wt.transpose(1, 0, 2).reshape(128, G * O)).astype(bf16)

    def wprep_m(w, G):
        # -> [p, (m, g, o')]: out-half-major so each m-chain DMAs separately
        wt = np.asarray(w, np.float32).T.reshape(G, 128, 2, 128)  # [g,p,m,o']
        return np.ascontiguousarray(
            wt.transpose(1, 2, 0, 3).reshape(128, G * O)).astype(bf16)

    # L0 symmetrization: channel c = 32*f1 + f2, z0[c] = x[f1]*x[f2] is
    # symmetric -> fold to 528 unordered pairs (pad to 640 = 5*128).
    ia, ib = _pair_indices()
    w0f = np.asarray(w0, np.float32).reshape(O, F, F)
    w0s = np.zeros((O, 640), np.float32)
    npair = len(ia)  # 528
    w0s[:, :npair] = w0f[:, ia, ib]
    off = ia != ib
    w0s[:, :npair][:, off] += w0f[:, ib[off], ia[off]]
    w0t = wprep(w0s, 5)
    w1t = wprep_m(w1, 32)
    w2t = wprep_m(w2, 32)
    common = {
        "w0t": w0t, "w1t": w1t, "w2t": w2t,
        "b0": np.ascontiguousarray(np.asarray(b0, np.float32)),
        "b1": np.ascontiguousarray(np.asarray(b1, np.float32)),
        "b2": np.ascontiguousarray(np.asarray(b2, np.float32)),
    }
    # pad pair index lists to 640 (padded channels have zero weight)
    iap = np.zeros(640, np.int64); iap[:len(ia)] = ia
    ibp = np.zeros(640, np.int64); ibp[:len(ib)] = ib
    in_maps = []
    for c in range(N_CORES):
        m = dict(common)
        xc = xb[c * BC:(c + 1) * BC]                     # [BC, F, E]
        # xtile[t, f, (b e)] = x[8t+b, f, e]
        xt = np.ascontiguousarray(
            xc.reshape(NT, NB, F, E).transpose(0, 2, 1, 3).reshape(NT, F, COLS))
        m["xtile"] = xt
        # xpa/xpb[t, p, (g c)] = xtile[t, idx[128g+p], c]  (pair operands)
        for nm, idx in (("xpa", iap), ("xpb", ibp)):
            g = xt[:, idx, :]                            # [NT, 640, COLS]
            m[nm] = np.ascontiguousarray(
                g.reshape(NT, 5, 128, COLS).transpose(0, 2, 1, 3)
                 .reshape(NT, 128, 5 * COLS))
        in_maps.append(m)
    return in_maps


def kernel(x, w0, b0, w1, b1, w2, b2, fc_w, fc_b, **kw):
    from concourse.bass_utils import run_bass_kernel_spmd

    if "nc" not in _CACHE:
        _CACHE["nc"] = _build()
    nc = _CACHE["nc"]
    in_maps = _prep_inputs(x, w0, b0, w1, b1, w2, b2, fc_w, fc_b)
    res = run_bass_kernel_spmd(nc, in_maps, list(range(N_CORES)))
    fcw = np.asarray(fc_w, np.float32).reshape(4, 128)
    ys = []
    for c in range(N_CORES):
        p = res.results[c]["pout"]  # [4, 128, BC]
        ys.append(np.einsum('cp,cpb->b', fcw, p.astype(np.float32)))
    out = np.concatenate(ys).reshape(B, 1).astype(np.float32)
    out = out + np.asarray(fc_b, np.float32).reshape(1, 1)
    return out
